# revision 1
# baseline (speedup 1.0000x reference)
"""Trainium2 Bass kernel: LSTM autoregressive decoder.

B=4096 batch data-parallel over 8 NeuronCores (512 rows/core). All state is
kept transposed on-chip (features on partitions, batch on the free dim) so the
recurrent matmuls need no per-step transposes:

  z^T[1024, n] = kernel^T @ x^T + rec_kernel^T @ h^T   (f32r matmuls, PSUM f32)
  gates: ACT sigmoid/tanh with per-partition bias, straight from PSUM
  c' = sig(f)*c + sig(i)*tanh(g); h' = sig(o)*tanh(c')  (DVE)
  y^T = relu(dense_w^T @ h' + db)                       (PE + DVE)

Weight layouts are pre-arranged on the host so every matmul lhsT is a plain
column slice. Gate bank m (0..7) = gate*2 + chunk, gate order (i,f,g,o),
feature u of a gate lives at (chunk=u//128, partition=u%128).
"""

import os
import sys

sys.path.insert(0, "/opt/trn_rl_repo")
os.environ.setdefault("MYCRO_LOCAL_CACHE", "1")

import numpy as np

import concourse.bacc as bacc
import concourse.bass as bass
import concourse.tile as tile
from concourse import bass_utils, mybir

f32 = mybir.dt.float32
f32r = mybir.dt.float32r
AF = mybir.ActivationFunctionType
ALU = mybir.AluOpType

B, U, O, S = 4096, 256, 128, 48
NCORES = 8
BL = B // NCORES  # 512 rows per core
N = BL            # free-dim (batch) tile

_build_cache = {}

# pool slot counts per tag — tunable; sim-swept
CFG = {"gate": 2, "th": 2, "t": 2, "c": 2, "h": 2, "y": 3, "z": 6, "yp": 2}


def build(steps=S):
    if steps in _build_cache:
        return _build_cache[steps]
    nc = bacc.Bacc("TRN2", target_bir_lowering=False)
    xT = nc.dram_tensor("xT", [O, N], f32, kind="ExternalInput")
    hT0 = nc.dram_tensor("hT0", [128, 2 * N], f32, kind="ExternalInput")
    cT0 = nc.dram_tensor("cT0", [128, 2 * N], f32, kind="ExternalInput")
    wk = nc.dram_tensor("wk", [128, 1024], f32, kind="ExternalInput")
    wr = nc.dram_tensor("wr", [128, 2048], f32, kind="ExternalInput")
    dwt = nc.dram_tensor("dwt", [128, 256], f32, kind="ExternalInput")
    bz = nc.dram_tensor("bz", [128, 8], f32, kind="ExternalInput")
    db = nc.dram_tensor("db", [128, 1], f32, kind="ExternalInput")
    yT = nc.dram_tensor("yT", [128, steps * N], f32, kind="ExternalOutput")

    with tile.TileContext(nc) as tc, \
         tc.tile_pool(name="consts", bufs=1) as cp, \
         tc.tile_pool(name="work", bufs=2) as wp, \
         tc.tile_pool(name="pz", bufs=CFG["z"], space="PSUM") as zp:

        # ---- weights: DMA fp32 in, DVE-round to f32r once
        wk_f = cp.tile([128, 1024], f32, tag="wk_f")
        wr_f = cp.tile([128, 2048], f32, tag="wr_f")
        dw_f = cp.tile([128, 256], f32, tag="dw_f")
        nc.sync.dma_start(out=wk_f, in_=wk[:, :])
        nc.sync.dma_start(out=wr_f, in_=wr[:, :])
        nc.sync.dma_start(out=dw_f, in_=dwt[:, :])
        wk_r = cp.tile([128, 1024], f32r, tag="wk_r")
        wr_r = cp.tile([128, 2048], f32r, tag="wr_r")
        dw_r = cp.tile([128, 256], f32r, tag="dw_r")
        nc.vector.tensor_copy(wk_r, wk_f)
        nc.vector.tensor_copy(wr_r, wr_f)
        nc.vector.tensor_copy(dw_r, dw_f)
        bz_t = cp.tile([128, 8], f32, tag="bz")
        db_t = cp.tile([128, 1], f32, tag="db")
        nc.sync.dma_start(out=bz_t, in_=bz[:, :])
        nc.sync.dma_start(out=db_t, in_=db[:, :])

        # ---- initial state (x,h rounded to f32r; c stays f32)
        x_f = cp.tile([O, N], f32, tag="x_f")
        h_f = cp.tile([128, 2 * N], f32, tag="h_f")
        nc.sync.dma_start(out=x_f, in_=xT[:, :])
        nc.sync.dma_start(out=h_f, in_=hT0[:, :])
        x_t = wp.tile([O, N], f32r, tag="y", bufs=CFG["y"])
        h_t = wp.tile([128, 2 * N], f32r, tag="h", bufs=CFG["h"])
        c_t = wp.tile([128, 2 * N], f32, tag="c", bufs=CFG["c"])
        nc.vector.tensor_copy(x_t, x_f)
        nc.vector.tensor_copy(h_t, h_f)
        nc.sync.dma_start(out=c_t, in_=cT0[:, :])

        GATE_FN = (AF.Sigmoid, AF.Sigmoid, AF.Tanh, AF.Sigmoid)  # i, f, g, o

        for s in range(steps):
            gt = [wp.tile([128, 2 * N], f32, tag=f"g{gi}", name=f"g{gi}_{s}",
                          bufs=CFG["gate"]) for gi in range(4)]
            cnew = wp.tile([128, 2 * N], f32, tag="c", name=f"c_{s}",
                           bufs=CFG["c"])

            def zbank(m):
                z_m = zp.tile([128, N], f32, tag="z", name=f"z{m}_{s}")
                lo, hi = m * 128, (m + 1) * 128
                nc.tensor.matmul(z_m, wr_r[:, lo:hi], h_t[:, 0:N],
                                 start=True, stop=False)
                nc.tensor.matmul(z_m, wr_r[:, 1024 + lo:1024 + hi],
                                 h_t[:, N:2 * N], start=False, stop=False)
                nc.tensor.matmul(z_m, wk_r[:, lo:hi], x_t,
                                 start=False, stop=True)
                gi, ch = m // 2, m % 2
                nc.scalar.activation(gt[gi][:, ch * N:(ch + 1) * N], z_m,
                                     GATE_FN[gi], bias=bz_t[:, m:m + 1])

            th = wp.tile([128, 2 * N], f32, tag="th", name=f"th_{s}",
                         bufs=CFG["th"])
            h_new = wp.tile([128, 2 * N], f32r, tag="h", name=f"h_{s}",
                            bufs=CFG["h"])
            yp = zp.tile([128, N], f32, tag="yp", name=f"yp_{s}",
                         bufs=CFG["yp"])

            def chunk_math(ch):
                cs = slice(ch * N, (ch + 1) * N)
                t1 = wp.tile([128, N], f32, tag="t1", name=f"t1_{s}_{ch}",
                             bufs=CFG["t"])
                t2 = wp.tile([128, N], f32, tag="t2", name=f"t2_{s}_{ch}",
                             bufs=CFG["t"])
                nc.vector.tensor_mul(t1, gt[1][:, cs], c_t[:, cs])
                nc.vector.tensor_mul(t2, gt[0][:, cs], gt[2][:, cs])
                nc.vector.tensor_add(cnew[:, cs], t1, t2)
                nc.scalar.activation(th[:, cs], cnew[:, cs], AF.Tanh)

            for m in (6, 7):      # o0, o1 first: sig(o) ready before tanh(c)
                zbank(m)
            for m in (0, 2, 4):   # i0, f0, g0
                zbank(m)
            chunk_math(0)
            for m in (1, 3, 5):   # i1, f1, g1
                zbank(m)
            chunk_math(1)

            for ch in (0, 1):     # h-muls after both chunks: no DVE head-block
                cs = slice(ch * N, (ch + 1) * N)
                nc.vector.tensor_mul(h_new[:, cs], gt[3][:, cs], th[:, cs])

            for ch in (0, 1):
                nc.tensor.matmul(yp, dw_r[:, ch * 128:(ch + 1) * 128],
                                 h_new[:, ch * N:(ch + 1) * N],
                                 start=(ch == 0), stop=(ch == 1))
            y_t = wp.tile([O, N], f32r, tag="y", bufs=CFG["y"], name=f"y_{s}")
            nc.vector.tensor_scalar(y_t, yp, db_t[:, 0:1], 0.0,
                                    op0=ALU.add, op1=ALU.max)
            nc.sync.dma_start(out=yT[:, s * N:(s + 1) * N],
                              in_=y_t[:, :].bitcast(f32))

            x_t, h_t, c_t = y_t, h_new, cnew

    if not nc.is_finalized():
        nc.finalize()
    _build_cache[steps] = nc
    return nc


def _prep_in_maps(last_input, h0, c0, kernel_w, rec_kernel, bias, dense_w, dense_b):
    f = np.float32
    last_input = np.asarray(last_input, dtype=f)
    h0 = np.asarray(h0, dtype=f)
    c0 = np.asarray(c0, dtype=f)
    kernel_w = np.asarray(kernel_w, dtype=f)
    rec_kernel = np.asarray(rec_kernel, dtype=f)
    bias = np.asarray(bias, dtype=f)
    dense_w = np.asarray(dense_w, dtype=f)
    dense_b = np.asarray(dense_b, dtype=f)

    wk = np.ascontiguousarray(kernel_w)                                   # [128,1024]
    wr = np.ascontiguousarray(
        rec_kernel.reshape(2, 128, 1024).transpose(1, 0, 2).reshape(128, 2048))
    dw = np.ascontiguousarray(
        dense_w.reshape(2, 128, 128).transpose(1, 0, 2).reshape(128, 256))
    bzv = np.ascontiguousarray(bias.reshape(8, 128).T)                    # [128,8]
    dbv = np.ascontiguousarray(dense_b.reshape(128, 1))

    def state_T(a):  # [BL,256] -> [128, 2*BL], chunk-major free dim
        return np.ascontiguousarray(
            a.T.reshape(2, 128, BL).transpose(1, 0, 2).reshape(128, 2 * BL))

    in_maps = []
    for c in range(NCORES):
        n0 = c * BL
        in_maps.append({
            "xT": np.ascontiguousarray(last_input[n0:n0 + BL].T),
            "hT0": state_T(h0[n0:n0 + BL]),
            "cT0": state_T(c0[n0:n0 + BL]),
            "wk": wk, "wr": wr, "dwt": dw, "bz": bzv, "db": dbv,
        })
    return in_maps


def _run(inputs, trace=False):
    steps = int(inputs.get("output_steps", S))
    nc = build(steps)
    in_maps = _prep_in_maps(
        inputs["last_input"], inputs["h0"], inputs["c0"], inputs["kernel"],
        inputs["rec_kernel"], inputs["bias"], inputs["dense_w"],
        inputs["dense_b"])
    res = bass_utils.run_bass_kernel_spmd(
        nc, in_maps, core_ids=list(range(NCORES)), trace=trace)
    shards = []
    for r in res.results:
        yTv = r["yT"].reshape(128, steps, BL)
        shards.append(yTv.transpose(2, 1, 0))  # [BL, steps, 128]
    full = np.ascontiguousarray(
        np.concatenate(shards, axis=0), dtype=np.float32)
    return full, res


def kernel(last_input, h0, c0, kernel, rec_kernel, bias, dense_w, dense_b,
           output_steps):
    full, _ = _run({
        "last_input": last_input, "h0": h0, "c0": c0, "kernel": kernel,
        "rec_kernel": rec_kernel, "bias": bias, "dense_w": dense_w,
        "dense_b": dense_b, "output_steps": int(output_steps),
    })
    return full

